# revision 36
# baseline (speedup 1.0000x reference)
"""Trainium2 Bass kernel for a 16-filter binarized 5x5 VALID conv.

Problem: x [B=32, C=6, H=512, W=512] f32; three grouped convs with
binarized 5x5 weights (channel subsets per output filter), concatenated
to out [32, 16, 508, 508] f32.

Mapping (per NeuronCore, data-parallel over batch, B/8 = 4 images each):
  conv == matmul with a banded block-Toeplitz stationary operand.
  The host pre-arranges each image into 32 window tiles of 20 rows
  (6 ch x 20 rows = 120 partitions; windows start at 16*wi for
  wi<31, and at 492 for the tail window) stored contiguously, so the
  whole image loads with ONE 3.9 MB contiguous DMA.

  Each window yields 16 output rows via TWO band-shifted weight sets
  (separate PSUM banks):
    hb=0: output rows w..w+7    (band over window rows 0..11)
    hb=1: output rows w+8..w+15 (band over window rows 8..19)
  Per band: M = 16 filters x 8 rows = 128 PSUM partitions (m = o*8+r),
  N = 504..508 columns; five matmuls (dx = kernel column; rhs =
  column-shifted slice of the window tile) accumulate into the bank.

  K=120 partitions per matmul matters: partial-K matmuls never
  un-throttle the PE HAM clock gate, so zero weight rows pad K to 120.

  Weights are binarized to sign(c)*alpha per filter; alpha is folded
  into the bf16 weight values (adds <0.4% rel err, well within budget).
  PSUM banks are evacuated with plain cast-copies (f32 -> bf16),
  alternating between the Vector and Scalar engines, into a staging
  tile covering 4 windows; each full staging tile is stored with one
  ~1 MB contiguous DMA.  The host converts bf16 -> f32 and scatters the
  band layout back to NCHW (both dtype/layout fixups, not compute).
  A 48-matmul warm-up burst on scratch data runs during the first input
  DMA so real matmuls start with the PE HAM clock gate fully open, and
  input DMAs alternate between the sync-HWDGE and gpsimd-SWDGE rings
  (each ring tops out near ~205 GB/s; two run in parallel).

Windows overlap at the image tail; overlapping rows are rewritten with
bitwise-identical values (same nonzero terms accumulated in the same
order).
"""

import numpy as np
import ml_dtypes

import concourse.bass as bass
import concourse.mybir as mybir
from concourse import bacc
from concourse import tile
from concourse.bass_utils import run_bass_kernel_spmd

MAPS3 = np.array([[0, 1, 2], [1, 2, 3], [2, 3, 4], [3, 4, 5], [0, 4, 5], [0, 1, 5]])
MAPS4 = np.array(
    [
        [0, 1, 2, 3],
        [1, 2, 3, 4],
        [2, 3, 4, 5],
        [0, 3, 4, 5],
        [0, 1, 4, 5],
        [0, 1, 2, 5],
        [0, 1, 3, 4],
        [1, 2, 4, 5],
        [0, 2, 3, 5],
    ]
)

C_IN = 6
N_OUT = 16
KH = KW = 5
R = 8  # output rows per band group
RH_WIN = 20  # input rows per SBUF window (2 bands, 4-row conv halo)
W_STRIDE = 16  # row stride between main windows
KDIM = C_IN * RH_WIN  # 120 contraction partitions
MP = 128  # PSUM partitions per matmul: m = o*8 + r
N_H = 2  # band positions per window
N_CORES = 8

H = W = 512
H_OUT = W_OUT = H - KH + 1  # 508
N_WIN = 32  # 31 main windows (stride 16) + tail window at row 492
TAIL_W = H - RH_WIN  # 492
GROUP_W = 4  # windows per output staging group / store DMA
N_GROUP = N_WIN // GROUP_W  # 8
IMG0_CHUNKS = [1, 1, 2, 2, 4, 4, 9, 9]  # first-image DMA chunks (fast start)
B_PER_CORE = 4

WIN_STARTS = [W_STRIDE * i for i in range(N_WIN - 1)] + [TAIL_W]


def _binarize_np(w):
    """Mirror reference.binarize in numpy fp32: sign matrix + per-filter alpha."""
    w = np.asarray(w, dtype=np.float32)
    m = w - w.mean(axis=1, keepdims=True)
    c = np.clip(m, -1.0, 1.0)
    alpha = np.abs(c).mean(axis=(1, 2, 3))
    return np.sign(c).astype(np.float32), alpha.astype(np.float32)


def _filter_table(w3, w4, w6):
    """Per output filter: (channel list, sign[ci,dy,dx] fp32, alpha)."""
    s3, a3 = _binarize_np(w3)
    s4, a4 = _binarize_np(w4)
    s6, a6 = _binarize_np(w6)
    table = []
    for o in range(6):
        table.append((list(MAPS3[o]), s3[o], a3[o]))
    for o in range(9):
        table.append((list(MAPS4[o]), s4[o], a4[o]))
    table.append((list(range(6)), s6[0], a6[0]))
    return table


def _build_weight_inputs(w3, w4, w6):
    """wm [KDIM, N_H*KW*128] bf16, alpha folded in: slice (hb*KW+dx) is the
    [120,128] stationary operand for band hb, kernel column dx."""
    table = _filter_table(w3, w4, w6)
    wm = np.zeros((KDIM, N_H, KW, MP), dtype=np.float32)
    for o, (chans, sgn, alpha) in enumerate(table):
        for r in range(R):
            m = o * R + r
            for hb in range(N_H):
                for ci, c in enumerate(chans):
                    for dy in range(KH):
                        k = c * RH_WIN + R * hb + r + dy
                        wm[k, hb, :, m] = sgn[ci, dy, :] * alpha
    return wm.reshape(KDIM, N_H * KW * MP).astype(ml_dtypes.bfloat16)


def build_nc(num_cores=N_CORES):
    """Build + compile the per-core Bass program."""
    f32 = mybir.dt.float32
    bf16 = mybir.dt.bfloat16
    wcols = N_WIN * W  # 16384 columns per window-tile row

    nc = bacc.Bacc(
        "TRN2",
        target_bir_lowering=False,
        debug=False,
        num_devices=num_cores,
    )
    wpre = N_H * KW * MP  # 1280 weight columns prefixed to image 0's tile
    x0_t = nc.dram_tensor("xb0", [KDIM, wpre + wcols], bf16, kind="ExternalInput")
    x_t = nc.dram_tensor(
        "xb", [B_PER_CORE - 1, KDIM, wcols], bf16, kind="ExternalInput"
    )
    out_t = nc.dram_tensor(
        "out",
        [B_PER_CORE, N_GROUP, MP, GROUP_W * N_H * W_OUT],
        bf16,
        kind="ExternalOutput",
    )

    with tile.TileContext(nc) as tc:
        with (
            tc.tile_pool(name="x0pool", bufs=1) as x0pool,
            tc.tile_pool(name="xpool", bufs=3) as xpool,
            tc.tile_pool(name="spool", bufs=3) as spool,
            tc.tile_pool(name="ppool", bufs=8, space="PSUM") as ppool,
        ):
            # Image 0's tile carries the weights as a column prefix, loaded as
            # part of its first input chunk (a separate weights DMA pays ~5us
            # of small-descriptor + ring-start latency).  All four image tiles
            # are persistent (no ring reuse), so the weights stay resident.
            x0t = x0pool.tile([KDIM, wpre + wcols], bf16, tag="x0")
            # Warm-up: keep the PE busy through the HAM activity window while
            # the first (weights-bearing) chunk is in flight, so real matmuls
            # start at the full 2.4 GHz clock.  Scratch PSUM, never read.
            warm = x0pool.tile([MP, MP], bf16, tag="warm")
            nc.vector.memset(warm[:], 0.0)
            pw = ppool.tile([MP, W_OUT], f32, tag="ps")
            for _ in range(38):
                nc.tensor.matmul(pw[:, :MP], warm[:], warm[:], start=True, stop=True)
            w0 = 0
            for ci, nw in enumerate(IMG0_CHUNKS):
                lo = wpre + w0 * W if ci else 0
                hi = wpre + (w0 + nw) * W
                if ci == 0:
                    # Partition-split the weights+window0 chunk across BOTH
                    # rings so it lands ~2us sooner; downstream chunk order
                    # on each ring is unchanged.
                    half = KDIM // 2
                    row = wpre + wcols
                    nc.sync.dma_start(
                        out=x0t[:half, lo:hi],
                        in_=bass.AP(x0_t, 0, [[row, half], [1, hi - lo]]),
                    )
                    nc.gpsimd.dma_start(
                        out=x0t[half:, lo:hi],
                        in_=bass.AP(x0_t, half * row, [[row, half], [1, hi - lo]]),
                    )
                else:
                    src = bass.AP(x0_t, lo, [[wpre + wcols, KDIM], [1, hi - lo]])
                    eng = nc.sync if ci % 2 == 0 else nc.gpsimd
                    eng.dma_start(out=x0t[:, lo:hi], in_=src)
                w0 += nw

            def load_image(b):
                """Load an image's window tiles in chunks, alternating between
                the sync-HWDGE and gpsimd-SWDGE DMA rings: a single ring tops
                out near ~205 GB/s, two in parallel roughly double that."""
                xt = xpool.tile([KDIM, wcols], bf16, tag="xt", name=f"xt_{b}")
                w0 = 0
                for ci, nw in enumerate([8, 8, 8, 8]):
                    c0, c1 = w0 * W, (w0 + nw) * W
                    src = bass.AP(
                        x_t,
                        (b - 1) * KDIM * wcols + c0,
                        [[wcols, KDIM], [1, c1 - c0]],
                    )
                    eng = nc.sync if ci % 2 == 0 else nc.gpsimd
                    eng.dma_start(out=xt[:, c0:c1], in_=src)
                    w0 += nw
                return xt

            wt = x0t  # weight slices live in image 0's column prefix
            xtiles = {0: x0t}
            for b in range(1, B_PER_CORE):
                xtiles[b] = load_image(b)

            for b in range(B_PER_CORE):
                xt = xtiles.pop(b)
                xoff = wpre if b == 0 else 0
                for g in range(N_GROUP):
                    st = spool.tile(
                        [MP, GROUP_W * N_H * W_OUT], bf16, tag="st", name=f"st_{b}_{g}"
                    )
                    for wl in range(GROUP_W):
                        wi = g * GROUP_W + wl
                        base = xoff + wi * W
                        for hb in range(N_H):
                            ps = ppool.tile([MP, W_OUT], f32, tag="ps")
                            for dx in range(KW):
                                sl = (hb * KW + dx) * MP
                                nc.tensor.matmul(
                                    ps[:],
                                    wt[:, sl : sl + MP],
                                    xt[:, base + dx : base + dx + W_OUT],
                                    start=(dx == 0),
                                    stop=(dx == KW - 1),
                                )
                            dst = st[:, (wl * N_H + hb) * W_OUT : (wl * N_H + hb + 1) * W_OUT]
                            final_band = (
                                b == B_PER_CORE - 1
                                and g == N_GROUP - 1
                                and wl == GROUP_W - 1
                                and hb == N_H - 1
                            )
                            if final_band:
                                # split across both engines so the last store
                                # can start as early as possible
                                half = W_OUT // 2
                                nc.vector.tensor_copy(dst[:, :half], ps[:, :half])
                                nc.scalar.copy(dst[:, half:], ps[:, half:])
                            elif hb == 0:
                                nc.vector.tensor_copy(dst, ps[:])
                            else:
                                nc.scalar.copy(dst, ps[:])
                    doff = (b * N_GROUP + g) * MP * GROUP_W * N_H * W_OUT
                    grow = GROUP_W * N_H * W_OUT
                    last = b == B_PER_CORE - 1 and g == N_GROUP - 1
                    halves = 8 if last else 1
                    hw_ = grow // halves
                    for hf in range(halves):
                        dstap = bass.AP(
                            out_t, doff + hf * hw_, [[grow, MP], [1, hw_]]
                        )
                        eng = nc.sync if hf % 2 == 1 else nc.scalar
                        eng.dma_start(
                            out=dstap, in_=st[:, hf * hw_ : (hf + 1) * hw_]
                        )

    nc.compile()
    return nc


_NC_CACHE = {}


def _get_nc():
    if "nc" not in _NC_CACHE:
        _NC_CACHE["nc"] = build_nc()
    return _NC_CACHE["nc"]


def _prep_inputs(x, w3, w4, w6):
    b = x.shape[0]
    assert b % N_CORES == 0 and x.shape[1:] == (C_IN, H, W)
    assert b // N_CORES == B_PER_CORE, "compiled program is per-core batch 4"
    wm = _build_weight_inputs(w3, w4, w6)
    xb = np.asarray(x).astype(ml_dtypes.bfloat16)
    # window-tile gather: [B, 6, 20, 32, 512] -> [B, 120, 32*512]
    starts = np.asarray(WIN_STARTS)
    idx = starts[None, :] + np.arange(RH_WIN)[:, None]  # [20, 32]
    xa = xb[:, :, idx, :]  # [B, 6, 20, 32, 512]
    xa = np.ascontiguousarray(xa).reshape(b, KDIM, N_WIN * W)
    bpc = b // N_CORES
    in_maps = []
    for i in range(N_CORES):
        blk = xa[i * bpc : (i + 1) * bpc]
        in_maps.append(
            {
                "xb0": np.ascontiguousarray(np.concatenate([wm, blk[0]], axis=1)),
                "xb": np.ascontiguousarray(blk[1:]),
            }
        )
    return bpc, in_maps


def _assemble(results, b_total):
    """[cores][4, 4, 128, 8*2*508] bf16 band tiles -> [B, 16, 508, 508] f32."""
    out = np.empty((b_total, N_OUT, H_OUT, W_OUT), dtype=np.float32)
    bpc = b_total // N_CORES
    for i, r in enumerate(results):
        o = np.asarray(r["out"]).astype(np.float32)
        o = o.reshape(bpc, N_GROUP, N_OUT, R, GROUP_W, N_H, W_OUT)
        for g in range(N_GROUP):
            for wl in range(GROUP_W):
                wi = g * GROUP_W + wl
                for hb in range(N_H):
                    rs = WIN_STARTS[wi] + R * hb
                    out[i * bpc : (i + 1) * bpc, :, rs : rs + R, :] = o[
                        :, g, :, :, wl, hb, :
                    ]
    return out


def run(x, w3, w4, w6, trace=False, **kw):
    b = x.shape[0]
    bpc, in_maps = _prep_inputs(x, w3, w4, w6)
    nc = _get_nc()
    res = run_bass_kernel_spmd(nc, in_maps, list(range(N_CORES)), trace=trace, **kw)
    return _assemble(res.results, b), res


def kernel(x, w3, w4, w6):
    out, _ = run(x, w3, w4, w6, trace=False)
    return out
